# revision 1
# baseline (speedup 1.0000x reference)
"""Chunked causal self-attention with RoPE on 8 Trainium2 NeuronCores.

Problem: B=4, L=4096, H=16, DH=DV=128, CHUNK=1024 (N=4 chunks).
RoPE is applied to q and k, then chunk-local causal attention per
(batch, chunk, head).

Sharding: heads are split across the 8 cores (2 heads/core), giving each
core B*N*heads_per_core = 32 independent (1024 x 128) attention problems.

Per-core kernel layout (all on-chip work in fp16 with fp32 PSUM accum):
  - Host pre-transposes q,k to d-major ("qT") layout and splits the RoPE
    halves, so every on-chip op is a full-128-partition operation and all
    DMA transfers are contiguous.
  - RoPE on DVE/GPSIMD: with A=[q1;q1], B=[q2;q2], C=[k1;k1], D=[k2;k2]
    (halves duplicated across partitions by the load DMA) and mixed
    cos/sin tables F=[c;s], E=[s;c]:
        mA=A*F=[q1c;q1s]  mB=B*E=[q2s;q2c]  -> q' = mA + sgn*mB
    via one scalar_tensor_tensor per tensor, sgn=[-1...;+1...].
  - S^T = k'.T @ q' per 128-row k-block (causal: only q >= k blocks),
    with the causal mask of the diagonal block pre-filled into PSUM via a
    matmul-copy of a -60000 upper-triangle tile (exact -inf masking).
  - exp on ScalarE (scale=1/sqrt(128) folded in), output fp16 P~.
  - out^T = sum_i V_i.T @ P~_i   (PSUM accumulation over k blocks)
  - den   = sum_k P~ via ones-weight matmul.
  - Normalization (out^T / den) and the final transpose back to natural
    layout happen on host (den and out^T are shipped out).
"""

import math
import os
import sys

import numpy as np

for _p in ("/opt/trn_rl_repo", "/root/.axon_site/_ro/trn_rl_repo"):
    if os.path.isdir(_p) and _p not in sys.path:
        sys.path.insert(0, _p)

import concourse.bass as bass  # noqa: E402
import concourse.tile as tile  # noqa: E402
from concourse import bass_utils, mybir  # noqa: E402

B, L, H, DH, DV = 4, 4096, 16, 128, 128
CHUNK = 1024
NCHUNK = L // CHUNK  # 4
ROPE_BASE = 10000.0
NCORES = 8
HPC = H // NCORES  # heads per core = 2
NPROB = B * HPC * NCHUNK  # 32 problems per core
HALF = DH // 2  # 64
NB = CHUNK // 128  # 8 k-blocks per chunk
SCALE = 1.0 / math.sqrt(DH)
MASK_NEG = -60000.0

F16 = mybir.dt.float16
F32 = mybir.dt.float32
AF = mybir.ActivationFunctionType
ALU = mybir.AluOpType


def _dup_ap(t, prob, rows, cols):
    """AP reading t[prob] (rows x cols) twice -> (2*rows, cols) partitions."""
    ap = t.ap()[prob]
    return bass.AP(
        tensor=ap.tensor,
        offset=ap.offset,
        ap=[[0, 2], [cols, rows], [1, cols]],
    )


def _pv_pieces(i):
    """Column pieces [a,b) of the 1024-wide q range for k-block i, each
    within a single 512-col PSUM bank."""
    q0 = 128 * i
    if q0 < 512:
        return [(q0, 512), (512, 1024)]
    return [(q0, 1024)]


def build_module(nprob=NPROB):
    from concourse import bacc

    nc = bacc.Bacc("TRN2", target_bir_lowering=False, debug=False)

    q1 = nc.dram_tensor("q1_in", (nprob, HALF, CHUNK), F16, kind="ExternalInput")
    q2 = nc.dram_tensor("q2_in", (nprob, HALF, CHUNK), F16, kind="ExternalInput")
    k1 = nc.dram_tensor("k1_in", (nprob, HALF, CHUNK), F16, kind="ExternalInput")
    k2 = nc.dram_tensor("k2_in", (nprob, HALF, CHUNK), F16, kind="ExternalInput")
    vv = nc.dram_tensor("v_in", (nprob, 128, CHUNK), F16, kind="ExternalInput")
    cs = nc.dram_tensor("cs_in", (NCHUNK, 2, 128, CHUNK), F16, kind="ExternalInput")
    msk = nc.dram_tensor("msk_in", (128, 128), F16, kind="ExternalInput")
    eye = nc.dram_tensor("eye_in", (128, 128), F16, kind="ExternalInput")
    ones = nc.dram_tensor("ones_in", (128, 1), F16, kind="ExternalInput")
    sgn = nc.dram_tensor("sgn_in", (128, 1), F16, kind="ExternalInput")

    outT = nc.dram_tensor("outT_out", (nprob, 128, CHUNK), F16, kind="ExternalOutput")
    den = nc.dram_tensor("den_out", (nprob, CHUNK), F16, kind="ExternalOutput")

    with tile.TileContext(nc) as tc:
        _body(tc, nprob, q1, q2, k1, k2, vv, cs, msk, eye, ones, sgn, outT, den)
    nc.compile()
    return nc


def _body(tc, nprob, q1, q2, k1, k2, vv, cs, msk, eye, ones, sgn, outT, den):
    from contextlib import ExitStack

    nc = tc.nc
    with ExitStack() as ctx:
        singles = ctx.enter_context(tc.tile_pool(name="singles", bufs=1))
        io = ctx.enter_context(tc.tile_pool(name="io", bufs=3))
        rope = ctx.enter_context(tc.tile_pool(name="rope", bufs=3))
        ptp = ctx.enter_context(tc.tile_pool(name="ptp", bufs=NB + 2))
        op = ctx.enter_context(tc.tile_pool(name="op", bufs=4))
        psS = ctx.enter_context(tc.tile_pool(name="psS", bufs=2, space="PSUM"))
        psOp = ctx.enter_context(tc.tile_pool(name="psO", bufs=1, space="PSUM"))
        psDp = ctx.enter_context(tc.tile_pool(name="psD", bufs=1, space="PSUM"))

        ftab, etab = [], []
        for n in range(NCHUNK):
            f = singles.tile([128, CHUNK], F16, tag=f"F{n}")
            nc.sync.dma_start(out=f, in_=cs.ap()[n, 0])
            ftab.append(f)
            e = singles.tile([128, CHUNK], F16, tag=f"E{n}")
            nc.sync.dma_start(out=e, in_=cs.ap()[n, 1])
            etab.append(e)
        msk_t = singles.tile([128, 128], F16, tag="msk")
        nc.sync.dma_start(out=msk_t, in_=msk.ap())
        eye_t = singles.tile([128, 128], F16, tag="eye")
        nc.sync.dma_start(out=eye_t, in_=eye.ap())
        ones_t = singles.tile([128, 1], F16, tag="ones")
        nc.sync.dma_start(out=ones_t, in_=ones.ap())
        sgn_t = singles.tile([128, 1], F16, tag="sgn")
        nc.sync.dma_start(out=sgn_t, in_=sgn.ap())

        # Touch each const tile once on DVE so later compute ops don't
        # carry multi-lane DMA waits (TT ISA struct has 1 wait slot).
        dummy = singles.tile([128, 1], F16, tag="dummy")
        for n in range(NCHUNK):
            nc.vector.tensor_copy(out=dummy, in_=ftab[n][:, 0:1])
            nc.vector.tensor_copy(out=dummy, in_=etab[n][:, 0:1])
        nc.vector.tensor_copy(out=dummy, in_=sgn_t)

        for p in range(nprob):
            n = p % NCHUNK

            ta = io.tile([128, CHUNK], F16, tag="A")
            nc.sync.dma_start(out=ta, in_=_dup_ap(q1, p, HALF, CHUNK))
            tb = io.tile([128, CHUNK], F16, tag="B")
            nc.sync.dma_start(out=tb, in_=_dup_ap(q2, p, HALF, CHUNK))
            tcc = io.tile([128, CHUNK], F16, tag="C")
            nc.sync.dma_start(out=tcc, in_=_dup_ap(k1, p, HALF, CHUNK))
            td = io.tile([128, CHUNK], F16, tag="D")
            nc.sync.dma_start(out=td, in_=_dup_ap(k2, p, HALF, CHUNK))
            tv = io.tile([128, CHUNK], F16, tag="V")
            nc.sync.dma_start(out=tv, in_=vv.ap()[p])

            # RoPE
            ma = rope.tile([128, CHUNK], F16, tag="mA")
            nc.vector.tensor_mul(ma, ta, ftab[n])
            mb = rope.tile([128, CHUNK], F16, tag="mB")
            nc.vector.tensor_mul(mb, tb, etab[n])
            mc = rope.tile([128, CHUNK], F16, tag="mC")
            nc.vector.tensor_mul(mc, tcc, ftab[n])
            md = rope.tile([128, CHUNK], F16, tag="mD")
            nc.vector.tensor_mul(md, td, etab[n])
            qp = rope.tile([128, CHUNK], F16, tag="qp")
            nc.vector.scalar_tensor_tensor(
                out=qp, in0=mb, scalar=sgn_t[:, :], in1=ma,
                op0=ALU.mult, op1=ALU.add,
            )
            kp = rope.tile([128, CHUNK], F16, tag="kp")
            nc.vector.scalar_tensor_tensor(
                out=kp, in0=md, scalar=sgn_t[:, :], in1=mc,
                op0=ALU.mult, op1=ALU.add,
            )

            # S^T per k-block row + exp
            pts = []
            for i in range(NB):
                q0 = 128 * i
                ps = psS.tile([128, CHUNK], F32, tag="psS")
                # exact causal masking: prefill diag block with upper-tri -inf
                nc.tensor.matmul(
                    ps[:, q0:q0 + 128], lhsT=eye_t, rhs=msk_t,
                    start=True, stop=False,
                )
                kblk = kp[:, q0:q0 + 128]
                nc.tensor.matmul(
                    ps[:, q0:q0 + 128], lhsT=kblk, rhs=qp[:, q0:q0 + 128],
                    start=False, stop=True,
                )
                rest = []
                if q0 + 128 < 512:
                    rest.append((q0 + 128, 512))
                if i <= 3:
                    rest.append((512, 1024))
                elif q0 + 128 < 1024:
                    rest.append((q0 + 128, 1024))
                for (a, b) in rest:
                    nc.tensor.matmul(
                        ps[:, a:b], lhsT=kblk, rhs=qp[:, a:b],
                        start=True, stop=True,
                    )
                pt = ptp.tile([128, CHUNK], F16, tag="pt")
                nc.scalar.activation(
                    out=pt[:, q0:CHUNK], in_=ps[:, q0:CHUNK], func=AF.Exp,
                    scale=SCALE,
                )
                pts.append(pt)

            # out^T accumulation over k blocks
            pso = psOp.tile([128, CHUNK], F32, tag="psO")
            for i in range(NB):
                vblk = tv[:, 128 * i:128 * i + 128]
                for (a, b) in _pv_pieces(i):
                    last = (i == 3 and b == 512) or (i == NB - 1)
                    nc.tensor.matmul(
                        pso[:, a:b], lhsT=vblk, rhs=pts[i][:, a:b],
                        start=(i == 0), stop=last,
                    )
            # denominator
            psd = psDp.tile([1, CHUNK], F32, tag="psD")
            for i in range(NB):
                for (a, b) in _pv_pieces(i):
                    last = (i == 3 and b == 512) or (i == NB - 1)
                    nc.tensor.matmul(
                        psd[:, a:b], lhsT=ones_t, rhs=pts[i][:, a:b],
                        start=(i == 0), stop=last,
                    )

            outf = op.tile([128, CHUNK], F16, tag="outf")
            nc.vector.tensor_copy(out=outf, in_=pso)
            nc.sync.dma_start(out=outT.ap()[p], in_=outf)
            dsb = op.tile([1, CHUNK], F16, tag="dsb")
            nc.vector.tensor_copy(out=dsb, in_=psd)
            nc.sync.dma_start(out=den.ap()[p], in_=dsb)


def _host_consts():
    half = HALF
    freqs = np.exp(np.arange(half, dtype=np.float64) * (-math.log(ROPE_BASE) / half))
    pos = np.arange(L, dtype=np.float64)
    ang = pos[:, None] * freqs[None, :]  # (L, 64)
    cos = np.cos(ang).astype(np.float32)
    sin = np.sin(ang).astype(np.float32)
    cs = np.zeros((NCHUNK, 2, 128, CHUNK), np.float16)
    for n in range(NCHUNK):
        c = cos[n * CHUNK:(n + 1) * CHUNK].T  # (64, 1024)
        s = sin[n * CHUNK:(n + 1) * CHUNK].T
        cs[n, 0, :half] = c  # F = [c; s]
        cs[n, 0, half:] = s
        cs[n, 1, :half] = s  # E = [s; c]
        cs[n, 1, half:] = c
    r = np.arange(128)
    msk = np.where(r[:, None] <= r[None, :], 0.0, MASK_NEG).astype(np.float16)
    eye = np.eye(128, dtype=np.float16)
    ones = np.ones((128, 1), np.float16)
    sgn = np.concatenate(
        [-np.ones((half, 1)), np.ones((half, 1))], axis=0
    ).astype(np.float16)
    return cs, msk, eye, ones, sgn


def _pack_core(qc, kc, vc):
    """qc,kc,vc: (B, L, HPC, 128) fp32 slices for one core -> input dict."""
    # (B, L, h, D) -> (B, h, N, C, D) -> d-major (B, h, N, D, C)
    def dmaj(x):
        return np.ascontiguousarray(
            x.transpose(0, 2, 1, 3)
            .reshape(B, HPC, NCHUNK, CHUNK, DH)
            .transpose(0, 1, 2, 4, 3)
        ).astype(np.float16)

    qT = dmaj(qc)  # (B, h, N, 128, 1024)
    kT = dmaj(kc)
    q1 = qT[:, :, :, :HALF].reshape(NPROB, HALF, CHUNK)
    q2 = qT[:, :, :, HALF:].reshape(NPROB, HALF, CHUNK)
    k1 = kT[:, :, :, :HALF].reshape(NPROB, HALF, CHUNK)
    k2 = kT[:, :, :, HALF:].reshape(NPROB, HALF, CHUNK)
    # v: (B, L, h, D) -> (B, h, N, NB, 128, D) -> (B, h, N, 128, NB, D)
    vp = (
        vc.transpose(0, 2, 1, 3)
        .reshape(B, HPC, NCHUNK, NB, 128, DV)
        .transpose(0, 1, 2, 4, 3, 5)
    )
    vp = np.ascontiguousarray(vp).astype(np.float16).reshape(NPROB, 128, CHUNK)
    return dict(q1_in=q1, q2_in=q2, k1_in=k1, k2_in=k2, v_in=vp)


_NC_CACHE = {}
LAST_RESULT = None


def _get_module(nprob=NPROB):
    if nprob not in _NC_CACHE:
        _NC_CACHE[nprob] = build_module(nprob)
    return _NC_CACHE[nprob]


def kernel(q, k, v):
    q = np.asarray(q, dtype=np.float32)
    k = np.asarray(k, dtype=np.float32)
    v = np.asarray(v, dtype=np.float32)

    cs, msk, eye, ones, sgn = _host_consts()
    consts = dict(cs_in=cs, msk_in=msk, eye_in=eye, ones_in=ones, sgn_in=sgn)

    in_maps = []
    for c in range(NCORES):
        hs = slice(HPC * c, HPC * (c + 1))
        m = _pack_core(q[:, :, hs], k[:, :, hs], v[:, :, hs])
        m.update(consts)
        in_maps.append(m)

    nc = _get_module(NPROB)
    trace = bool(int(os.environ.get("KERNEL_TRACE", "0")))
    res = bass_utils.run_bass_kernel_spmd(
        nc, in_maps, core_ids=list(range(NCORES)), trace=trace
    )
    global LAST_RESULT
    LAST_RESULT = res

    out = np.empty((B, L, H, DV), np.float32)
    for c in range(NCORES):
        ot = res.results[c]["outT_out"].astype(np.float32)  # (32, 128dv, 1024q)
        dn = res.results[c]["den_out"].astype(np.float32)  # (32, 1024q)
        o = ot / dn[:, None, :]
        # (B, h, N, dv, q) -> (B, N, q, h, dv)
        o = o.reshape(B, HPC, NCHUNK, DV, CHUNK).transpose(0, 2, 4, 1, 3)
        out[:, :, HPC * c:HPC * (c + 1)] = o.reshape(B, L, HPC, DV)
    return out



# revision 10
# speedup vs baseline: 2.1907x; 2.1907x over previous
"""Chunked causal self-attention with RoPE on 8 Trainium2 NeuronCores.

Problem: B=4, L=4096, H=16, DH=DV=128, CHUNK=1024 (N=4 chunks).
RoPE on q,k then chunk-local causal attention per (batch, chunk, head).

Sharding: heads split across 8 cores (2 heads/core) -> 32 independent
(1024 x 1024, d=128) attention problems per core, grouped 4-per-(b,h)
so one RoPE table pass covers a whole group.

v2 design notes (from trace analysis of v1):
  - All inputs packed d-major as (128, NPROB*1024) fp16 so every group
    load is one DMA with 128 x 8KB descriptors; a single sync-queue
    stream sustains ~400GB/s this way.
  - PE work: per problem 12 score matmuls (4608 cols), 12 PV matmuls
    (4608 cols), 2 denominator matmuls over an fp16 block-sum R (1024
    cols). No mask matmuls: causal diag masking is a DVE multiply with
    an upper-tri 0/1 tile on the fp16 P-tile.
  - Software pipeline one problem deep (PE order: PV(p-1), den(p-1),
    scores(p)) keeps the PE continuously busy so it reaches the 2.4GHz
    p-state instead of 1.2GHz.
  - exp on ScalarE; narrow blocks are paired into shared PSUM tiles so
    there are 6 activation instructions per problem instead of 8.
  - Normalization (outT/den) and final layout transposes on host.
"""

import math
import os
import sys

import numpy as np

for _p in ("/opt/trn_rl_repo", "/root/.axon_site/_ro/trn_rl_repo"):
    if os.path.isdir(_p) and _p not in sys.path:
        sys.path.insert(0, _p)

import concourse.bass as bass  # noqa: E402
import concourse.tile as tile  # noqa: E402
from concourse import bass_utils, mybir  # noqa: E402

B, L, H, DH, DV = 4, 4096, 16, 128, 128
CHUNK = 1024
NCHUNK = L // CHUNK  # 4
ROPE_BASE = 10000.0
NCORES = 8
HPC = H // NCORES  # 2 heads per core
NPROB = B * HPC * NCHUNK  # 32 problems per core
NG = B * HPC  # 8 groups of 4 chunks
HALF = DH // 2  # 64
NB = CHUNK // 128  # 8 k-blocks
SCALE = 1.0 / math.sqrt(DH)

F16 = mybir.dt.float16
F32 = mybir.dt.float32
AF = mybir.ActivationFunctionType

# exp "units": list of (psS column offset, block) pairs per unit.
# Blocks 0-3 get their own [128,1024] PSUM tile; (4,5) and (6,7) share.
# Each entry: (unit_tag, [(block, tile_col_off)])
UNITS = [
    ("u0", [(0, 0)]),
    ("u1", [(1, 128)]),
    ("u2", [(2, 256)]),
    ("u3", [(3, 384)]),
    ("u45", [(4, 0), (5, 512)]),
    ("u67", [(6, 0), (7, 256)]),
]
# For single blocks the tile col offset equals q0 so piece splits at 512
# stay bank-aligned. For paired units each block's region starts at a
# bank boundary (0 / 512) or stays within one bank.


def _block_region(b):
    """absolute q range covered for k-block b (causal)."""
    return 128 * b, CHUNK


def build_module(nprob=NPROB):
    from concourse import bacc

    nc = bacc.Bacc("TRN2", target_bir_lowering=False, debug=False)

    qT = nc.dram_tensor("qT_in", (128, nprob * CHUNK), F16, kind="ExternalInput")
    kT = nc.dram_tensor("kT_in", (128, nprob * CHUNK), F16, kind="ExternalInput")
    vT = nc.dram_tensor("vT_in", (128, nprob * CHUNK), F16, kind="ExternalInput")
    c2 = nc.dram_tensor("c2_in", (128, L), F16, kind="ExternalInput")
    s2 = nc.dram_tensor("s2_in", (128, L), F16, kind="ExternalInput")
    tri = nc.dram_tensor("tri_in", (128, 128), F16, kind="ExternalInput")
    ones = nc.dram_tensor("ones_in", (128, 1), F16, kind="ExternalInput")

    outT = nc.dram_tensor("outT_out", (128, nprob * CHUNK), F16, kind="ExternalOutput")
    den = nc.dram_tensor(
        "den_out", (nprob // NCHUNK, NCHUNK * CHUNK), F16, kind="ExternalOutput"
    )

    with tile.TileContext(nc) as tc:
        _body(tc, nprob, qT, kT, vT, c2, s2, tri, ones, outT, den)
    nc.compile()
    return nc


def _body(tc, nprob, qT, kT, vT, c2, s2, tri, ones, outT, den):
    from contextlib import ExitStack

    nc = tc.nc
    ngroups = nprob // NCHUNK
    GW = NCHUNK * CHUNK  # group width: 4096 cols

    with ExitStack() as ctx:
        consts = ctx.enter_context(tc.tile_pool(name="consts", bufs=1))
        ing = ctx.enter_context(tc.tile_pool(name="ing", bufs=2))
        qkp = ctx.enter_context(tc.tile_pool(name="qkp", bufs=2))
        ptp = ctx.enter_context(tc.tile_pool(name="ptp", bufs=16))
        rp = ctx.enter_context(tc.tile_pool(name="rp", bufs=2))
        outp = ctx.enter_context(tc.tile_pool(name="outp", bufs=2))
        dnp = ctx.enter_context(tc.tile_pool(name="dnp", bufs=2))
        psSp = ctx.enter_context(tc.tile_pool(name="psS", bufs=2, space="PSUM"))
        psOp = ctx.enter_context(tc.tile_pool(name="psO", bufs=1, space="PSUM"))
        psDp = ctx.enter_context(tc.tile_pool(name="psD", bufs=1, space="PSUM"))

        c2_t = consts.tile([128, L], F16, tag="c2")
        nc.sync.dma_start(out=c2_t, in_=c2.ap())
        s2_t = consts.tile([128, L], F16, tag="s2")
        nc.sync.dma_start(out=s2_t, in_=s2.ap())
        tri_t = consts.tile([128, 128], F16, tag="tri")
        nc.sync.dma_start(out=tri_t, in_=tri.ap())
        ones_t = consts.tile([128, 1], F16, tag="ones")
        nc.sync.dma_start(out=ones_t, in_=ones.ap())

        # touch consts once so compute ops don't carry extra DMA waits
        dummy = consts.tile([128, 1], F16, tag="dummy")
        nc.vector.tensor_copy(out=dummy, in_=c2_t[:, 0:1])
        nc.vector.tensor_copy(out=dummy, in_=s2_t[:, 0:1])
        nc.vector.tensor_copy(out=dummy, in_=tri_t[:, 0:1])
        nc.vector.tensor_copy(out=dummy, in_=ones_t)

        state = {}  # per live problem: pt tiles, R, group tiles

        def emit_loads_rope(g):
            qg = ing.tile([128, GW], F16, tag="qg")
            nc.sync.dma_start(out=qg, in_=qT.ap()[:, g * GW:(g + 1) * GW])
            kg = ing.tile([128, GW], F16, tag="kg")
            nc.sync.dma_start(out=kg, in_=kT.ap()[:, g * GW:(g + 1) * GW])
            vg = ing.tile([128, GW], F16, tag="vg")
            nc.sync.dma_start(out=vg, in_=vT.ap()[:, g * GW:(g + 1) * GW])
            # swapped-half copies [x2; x1] via SBUF->SBUF DMA (partition
            # shift is illegal for 2-input DVE ops but fine for DMA)
            qsw = ing.tile([128, GW], F16, tag="qsw")
            nc.sync.dma_start(out=qsw[0:HALF, :], in_=qg[HALF:128, :])
            nc.sync.dma_start(out=qsw[HALF:128, :], in_=qg[0:HALF, :])
            ksw = ing.tile([128, GW], F16, tag="ksw")
            nc.sync.dma_start(out=ksw[0:HALF, :], in_=kg[HALF:128, :])
            nc.sync.dma_start(out=ksw[HALF:128, :], in_=kg[0:HALF, :])

            def rope(src, swp, tag):
                # src *= [c;c]; swp *= [-s;s]; dst = src + swp  (in place)
                nc.vector.tensor_mul(src, src, c2_t)
                nc.vector.tensor_mul(swp, swp, s2_t)
                dst = qkp.tile([128, GW], F16, name=tag, tag=tag)
                nc.vector.tensor_add(dst, src, swp)
                return dst

            qp = rope(qg, qsw, "qp")
            kp = rope(kg, ksw, "kp")
            state[("grp", g)] = (vg, qp, kp)

        def emit_scores_exp(p):
            g, pi = divmod(p, NCHUNK)
            vg, qp, kp = state[("grp", g)]
            poff = pi * CHUNK
            pts = {}
            for tag, blocks in UNITS:
                ps = psSp.tile([128, CHUNK], F32, tag="psS")
                # score matmuls for each block in this unit
                for b, off in blocks:
                    q0, q1 = _block_region(b)
                    kblk = kp[:, poff + 128 * b: poff + 128 * (b + 1)]
                    # pieces of [q0,q1) split at bank boundaries rel. tile
                    # tile col of abs q is (q - q0 + off)
                    a = q0
                    while a < q1:
                        # bank boundary in tile coords
                        ta = a - q0 + off
                        bank_end = ((ta // 512) + 1) * 512
                        e = min(q1, a + (bank_end - ta))
                        nc.tensor.matmul(
                            ps[:, ta:ta + (e - a)],
                            lhsT=kblk,
                            rhs=qp[:, poff + a: poff + e],
                            start=True, stop=True,
                        )
                        a = e
                pt = ptp.tile([128, CHUNK], F16, tag="pt")
                # one exp over the full used span of this unit tile
                lo_off = min(off for b, off in blocks)
                hi_off = max(off + (CHUNK - 128 * b) for b, off in blocks)
                nc.scalar.activation(
                    out=pt[:, lo_off:hi_off], in_=ps[:, lo_off:hi_off],
                    func=AF.Exp, scale=SCALE,
                )
                # diag masks on DVE (in place), then R-chain handled below
                for b, off in blocks:
                    nc.vector.tensor_mul(
                        pt[:, off:off + 128], pt[:, off:off + 128], tri_t
                    )
                    pts[b] = (pt, off)
            # R = sum over blocks of P-tilde (fp16, aligned on abs q)
            R = rp.tile([128, CHUNK], F16, tag="R")
            pt0, off0 = pts[0]
            nc.vector.tensor_copy(out=R, in_=pt0[:, off0:off0 + CHUNK])
            for b in range(1, NB):
                ptb, offb = pts[b]
                q0, q1 = _block_region(b)
                nc.vector.tensor_add(
                    R[:, q0:q1], R[:, q0:q1], ptb[:, offb:offb + (q1 - q0)]
                )
            state[("pt", p)] = pts
            state[("R", p)] = R

        def emit_pv_den(p):
            g, pi = divmod(p, NCHUNK)
            vg, qp, kp = state[("grp", g)]
            poff = pi * CHUNK
            pts = state.pop(("pt", p))
            R = state.pop(("R", p))
            pso = psOp.tile([128, CHUNK], F32, tag="psO")
            # accumulate over blocks; per psum bank the last writer stops
            last_in_bank = {0: 3, 1: NB - 1}  # bank0 cols [0,512): blocks 0..3
            for b in range(NB):
                q0, q1 = _block_region(b)
                ptb, offb = pts[b]
                vblk = vg[:, poff + 128 * b: poff + 128 * (b + 1)]
                a = q0
                while a < q1:
                    bank = a // 512
                    e = min(q1, (bank + 1) * 512)
                    nc.tensor.matmul(
                        pso[:, a:e],
                        lhsT=vblk,
                        rhs=ptb[:, offb + (a - q0): offb + (e - q0)],
                        start=(b == 0),
                        stop=(b == last_in_bank[bank]),
                    )
                    a = e
            psd = psDp.tile([1, CHUNK], F32, tag="psD")
            nc.tensor.matmul(psd[:, 0:512], lhsT=ones_t, rhs=R[:, 0:512],
                             start=True, stop=True)
            nc.tensor.matmul(psd[:, 512:CHUNK], lhsT=ones_t, rhs=R[:, 512:CHUNK],
                             start=True, stop=True)

            # drain psO/psD to SBUF fp16 (DVE), batched per group
            if pi == 0:
                state[("outg", g)] = outp.tile([128, GW], F16, name="outg", tag="outg")
                state[("deng", g)] = dnp.tile([1, GW], F16, name="deng", tag="deng")
            outg = state[("outg", g)]
            deng = state[("deng", g)]
            nc.vector.tensor_copy(out=outg[:, poff:poff + CHUNK], in_=pso)
            nc.vector.tensor_copy(out=deng[:, poff:poff + CHUNK], in_=psd)
            if pi == NCHUNK - 1:
                nc.sync.dma_start(
                    out=outT.ap()[:, g * GW:(g + 1) * GW], in_=outg
                )
                nc.sync.dma_start(out=den.ap()[g], in_=deng)
                state.pop(("outg", g))
                state.pop(("deng", g))
                state.pop(("grp", g))

        # main software-pipelined loop
        for p in range(nprob + 1):
            if p > 0:
                emit_pv_den(p - 1)
            if p < nprob:
                if p == 0:
                    emit_loads_rope(0)
                if p % NCHUNK == 1 and (p // NCHUNK) + 1 < ngroups:
                    emit_loads_rope(p // NCHUNK + 1)
                emit_scores_exp(p)


def _host_consts():
    freqs = np.exp(np.arange(HALF, dtype=np.float64) * (-math.log(ROPE_BASE) / HALF))
    pos = np.arange(L, dtype=np.float64)
    ang = pos[:, None] * freqs[None, :]  # (L, 64)
    cos = np.cos(ang)
    sin = np.sin(ang)
    # C2 = [c;c]; with qsw = [q2;q1] the S table is [-s;+s]:
    #   q'[0:64]  = q1*c + q2*(-s);  q'[64:128] = q2*c + q1*(+s)
    c2 = np.concatenate([cos.T, cos.T], axis=0).astype(np.float16)  # (128, L)
    s2 = np.concatenate([-sin.T, sin.T], axis=0).astype(np.float16)
    r = np.arange(128)
    tri = (r[None, :] >= r[:, None]).astype(np.float16)  # keep q >= k
    ones = np.ones((128, 1), np.float16)
    return c2, s2, tri, ones


def _pack_core(qc, kc, vc):
    """qc,kc,vc: (B, L, HPC, 128) fp32 -> d-major (128, NPROB*1024) fp16."""

    def dmaj(x):
        # (B, L, h, D) -> (b, h, n, j, d) -> (d, b, h, n, j)
        a = x.transpose(0, 2, 1, 3).reshape(B, HPC, NCHUNK, CHUNK, DH)
        a = a.transpose(4, 0, 1, 2, 3).reshape(DH, NPROB * CHUNK)
        return np.ascontiguousarray(a).astype(np.float16)

    # v: partition = k-within-block, cols = (b,h,n, block, dv)
    a = vc.transpose(0, 2, 1, 3).reshape(B, HPC, NCHUNK, NB, 128, DV)
    a = a.transpose(4, 0, 1, 2, 3, 5).reshape(128, NPROB * CHUNK)
    vp = np.ascontiguousarray(a).astype(np.float16)
    return dict(qT_in=dmaj(qc), kT_in=dmaj(kc), vT_in=vp)


_NC_CACHE = {}
LAST_RESULT = None


def _get_module(nprob=NPROB):
    if nprob not in _NC_CACHE:
        _NC_CACHE[nprob] = build_module(nprob)
    return _NC_CACHE[nprob]


def kernel(q, k, v):
    q = np.asarray(q, dtype=np.float32)
    k = np.asarray(k, dtype=np.float32)
    v = np.asarray(v, dtype=np.float32)

    c2, s2, tri, ones = _host_consts()
    consts = dict(c2_in=c2, s2_in=s2, tri_in=tri, ones_in=ones)

    in_maps = []
    for c in range(NCORES):
        hs = slice(HPC * c, HPC * (c + 1))
        m = _pack_core(q[:, :, hs], k[:, :, hs], v[:, :, hs])
        m.update(consts)
        in_maps.append(m)

    nc = _get_module(NPROB)
    trace = bool(int(os.environ.get("KERNEL_TRACE", "0")))
    res = bass_utils.run_bass_kernel_spmd(
        nc, in_maps, core_ids=list(range(NCORES)), trace=trace
    )
    global LAST_RESULT
    LAST_RESULT = res

    out = np.empty((B, L, H, DV), np.float32)
    for c in range(NCORES):
        ot = res.results[c]["outT_out"].astype(np.float32)  # (128dv, 32*1024)
        dn = res.results[c]["den_out"].astype(np.float32).reshape(-1)  # (32*1024,)
        o = ot / dn[None, :]
        # (dv, b, h, n, j) -> (b, n*j=L, h, dv)
        o = o.reshape(DV, B, HPC, NCHUNK, CHUNK).transpose(1, 3, 4, 2, 0)
        out[:, :, HPC * c:HPC * (c + 1)] = o.reshape(B, L, HPC, DV)
    return out
